# revision 1
# baseline (speedup 1.0000x reference)
"""Trainium2 Bass kernel for nn_LinearPerBlockQuant (per-block fake-quant linear).

  out = fake_quant(x; a_scales, a_zeros) @ fake_quant(W; w_scales, w_zeros).T + bias

Shapes: x (1024, 4096) f32, W (4096, 4096), block size 4 along IN,
w_scales/w_zeros (4096, 1024), a_scales/a_zeros (1024,), bias (4096,).

Sharding: column-parallel over 8 NeuronCores -- each core owns 512 output
features (W rows, scales, bias shards); x is replicated. Host concatenates
the 8 (512, 1024) partial outputs and transposes.

Device-side per core (strip-streaming design):
  - x and W both arrive pre-transposed + block-permuted on the k axis:
      xT[r*1024+kb, b] = x[b, 4*kb+r];  wT[r*1024+kb, o] = W[o, 4*kb+r]
    so k is the partition dim everywhere and per-k activation quant
    scales are per-partition scalars (ACT scale/bias fusion). Weight
    scales arrive transposed (wsT/wzT (1024, 512)), so in a k-strip the
    per-(o, block) scales are dense (128, 512) tensor operands shared by
    the 4 strips of one kb-octave (prefetched one octave ahead).
  - quant: q = sat_u8(v * (1/s) + z)  (HW u8 conversion = round-half-even +
    saturate == clip(round(.), 0, 255), HW-verified)
  - x path on ACT: f32 -> u8 (quant), u8 -> bf16 (dequant), both with
    per-partition scale/bias fusion. W path: gpsimd mult + DVE add/sub/mult,
    result bf16. bf16 keeps full qx (64KB/part) + wqT (32KB/part) resident.
  - matmul: 8 psum tiles (128, 512) = all 8 banks, one accumulation chain
    per (ot, b-half), accumulated strip-by-strip as data lands (bf16 =
    1 cycle/row).
  - drain: bias added on psum drain, alternating ACT/DVE so the 8 drains
    run in parallel pairs; output written bf16 (halves output DMA), one
    DMA per (ot, b-half) fired straight after its drain.
"""
import os
import numpy as np
from contextlib import ExitStack

import concourse.bass as bass
import concourse.tile as tile
from concourse import bacc, mybir
from concourse.bass_utils import run_bass_kernel_spmd
from concourse.masks import make_identity

F32 = mybir.dt.float32
BF16 = mybir.dt.bfloat16
U8 = mybir.dt.uint8
OP = mybir.AluOpType
AF = mybir.ActivationFunctionType

B, IN, OUT, BS = 1024, 4096, 4096, 4
NCORES = 8
OSH = OUT // NCORES          # 512 out-features per core
NB = IN // BS                # 1024 blocks along IN
NKT = IN // 128              # 32 k-strips of 128
NOCT = 8                     # kb-octaves (128 kb values each)
NOT = OSH // 128             # 4 output-feature tiles per core
OUT_BF16 = True              # write output as bf16 (halves output DMA)
# hold PE back until strip DUMMY_STRIP's qx is ready; DUMMY_NCHAIN of the 8
# psum chains are held (dummy transpose creates a WAR dep on the psum tile)
DUMMY_STRIP = int(os.environ.get("LPBQ_DUMMY_STRIP", "-1"))
DUMMY_NCHAIN = int(os.environ.get("LPBQ_DUMMY_NCHAIN", "1"))
if DUMMY_STRIP < 0:
    DUMMY_STRIP = None
# dep-free add-zero matmuls emitted before each strip's real matmuls: they
# run while PE would otherwise starve, so the p-state ramp never resets and
# the (PE-paced) endgame runs at full clock instead of 1.2GHz
FILLERS = int(os.environ.get("LPBQ_FILLERS", "0"))
FILL_FROM = int(os.environ.get("LPBQ_FILL_FROM", "26"))
NCST = 2 * NOCT + NOT        # asc | az | bias columns

_CACHE = {}


def _build_nc():
    nc = bacc.Bacc("TRN2", target_bir_lowering=False, debug=False)

    xT_d = nc.dram_tensor("xT", [IN, B], F32, kind="ExternalInput").ap()
    wT_d = nc.dram_tensor("wT", [IN, OSH], F32, kind="ExternalInput").ap()
    wsT_d = nc.dram_tensor("wsT", [NB, OSH], F32, kind="ExternalInput").ap()
    wzT_d = nc.dram_tensor("wzT", [NB, OSH], F32, kind="ExternalInput").ap()
    cst_d = nc.dram_tensor("cst", [128, NCST], F32, kind="ExternalInput").ap()
    out_dt = BF16 if OUT_BF16 else F32
    out_d = nc.dram_tensor("out", [OSH, B], out_dt, kind="ExternalOutput").ap()

    with tile.TileContext(nc) as tc, ExitStack() as ctx:
        const = ctx.enter_context(tc.tile_pool(name="const", bufs=1))
        big = ctx.enter_context(tc.tile_pool(name="big", bufs=1))
        xrp = ctx.enter_context(tc.tile_pool(name="xr", bufs=5))
        q8p = ctx.enter_context(tc.tile_pool(name="q8", bufs=3))
        wtp = ctx.enter_context(tc.tile_pool(name="wt", bufs=4))
        wsp = ctx.enter_context(tc.tile_pool(name="wsp", bufs=3))
        wzp = ctx.enter_context(tc.tile_pool(name="wzp", bufs=3))
        rwsp = ctx.enter_context(tc.tile_pool(name="rws", bufs=3))
        tdp = ctx.enter_context(tc.tile_pool(name="td", bufs=4))
        q8wp = ctx.enter_context(tc.tile_pool(name="q8w", bufs=3))
        outp = ctx.enter_context(tc.tile_pool(name="outp", bufs=4))
        psm = ctx.enter_context(tc.tile_pool(name="psm", bufs=1, space="PSUM"))

        # ---- first strip's big DMAs before anything small: fill the pipe ----
        wt0 = wtp.tile([128, OSH], F32, tag="wt")
        nc.sync.dma_start(wt0[:], wT_d[0:128, :])
        xr0 = xrp.tile([128, B], F32, tag="xr")
        nc.sync.dma_start(xr0[:], xT_d[0:128, :])

        # dummy activation with no data deps: hoists the implicit
        # LoadActFuncSet (1.28us) to t~0 instead of before the first quant
        scr_t = const.tile([128, 1], F32)
        nc.vector.memset(scr_t[:], 0.0)
        nc.scalar.activation(scr_t[:], scr_t[:], AF.Identity,
                             bias=0.0, scale=1.0)

        # ---- constants: asc | az | bias in one DMA ----
        cst_t = const.tile([128, NCST], F32)
        nc.sync.dma_start(cst_t[:], cst_d)
        asc_t = cst_t[:, 0:NOCT]
        az_t = cst_t[:, NOCT:2 * NOCT]
        bias_t = cst_t[:, 2 * NOCT:]
        ras_t = const.tile([128, NOCT], F32)
        nc.vector.reciprocal(ras_t[:], asc_t)
        # nzsa = -(za * sa)
        nzsa_t = const.tile([128, NOCT], F32)
        nc.vector.scalar_tensor_tensor(nzsa_t[:], az_t, -1.0, asc_t,
                                       OP.mult, OP.mult)
        ident = None
        if DUMMY_STRIP is not None:
            ident = const.tile([128, 128], BF16)
            make_identity(nc, ident[:])

        # ---- resident big tensors ----
        qx_t = big.tile([128, NKT * B], BF16)     # dequant activations
        wq_t = big.tile([128, NKT * OSH], BF16)   # dequant transposed weights

        # 8 psum accumulators: (ot, b-half), each (128, 512) = one bank
        pacc = [psm.tile([128, 512], F32, name=f"pacc{j}") for j in range(8)]

        z_t = None
        if FILLERS:
            z_t = const.tile([128, 512], BF16)
            nc.vector.memset(z_t[:], 0.0)

        dummy_emitted = [DUMMY_STRIP is None]
        fill_cnt = [0]

        def emit_fillers(n):
            for _ in range(n):
                j = fill_cnt[0] % 8
                fill_cnt[0] += 1
                nc.tensor.matmul(pacc[j][:], z_t[:, 0:128], z_t[:],
                                 start=False, stop=False)

        def emit_scales(oct_):
            ws_t = wsp.tile([128, OSH], F32, tag="ws")
            nc.sync.dma_start(ws_t[:], wsT_d[128 * oct_:128 * (oct_ + 1), :])
            wz_t = wzp.tile([128, OSH], F32, tag="wz")
            nc.sync.dma_start(wz_t[:], wzT_d[128 * oct_:128 * (oct_ + 1), :])
            rws_t = rwsp.tile([128, OSH], F32, tag="rws")
            nc.vector.reciprocal_approx_fast(rws_t[:], ws_t[:])
            return ws_t, wz_t, rws_t

        def emit_strip(i, oct_, r, scales, wx0=None, halves=False):
            kt = r * NOCT + oct_
            ws_t, wz_t, rws_t = scales
            # --- DMAs (w first: its chain is one hop longer) ---
            if wx0 is not None:
                wt_i, xr_i = wx0
            elif halves:
                # split the final strip's DMAs so each half lands (and its
                # dependent chain starts) one half-transfer earlier
                wt_i = wtp.tile([128, OSH], F32, tag="wt")
                xr_i = xrp.tile([128, B], F32, tag="xr")
                for h in range(2):
                    ws_ = slice(h * (OSH // 2), (h + 1) * (OSH // 2))
                    nc.sync.dma_start(wt_i[:, ws_],
                                      wT_d[128 * kt:128 * (kt + 1), ws_])
                for h in range(2):
                    xs_ = slice(h * (B // 2), (h + 1) * (B // 2))
                    nc.sync.dma_start(xr_i[:, xs_],
                                      xT_d[128 * kt:128 * (kt + 1), xs_])
            else:
                wt_i = wtp.tile([128, OSH], F32, tag="wt")
                nc.sync.dma_start(wt_i[:], wT_d[128 * kt:128 * (kt + 1), :])
                xr_i = xrp.tile([128, B], F32, tag="xr")
                nc.sync.dma_start(xr_i[:], xT_d[128 * kt:128 * (kt + 1), :])
            # --- W chain: t = w*rws (Pool); q8w = u8(t+wz); d = q8w-wz;
            #     wq = bf16(d*ws) --- (half-split on the last strip so the
            # tail-chain latency after the final DMA is ~halved)
            t_t = tdp.tile([128, OSH], F32, tag="t")
            q8w = q8wp.tile([128, OSH], U8, tag="q8w")
            d_t = tdp.tile([128, OSH], F32, tag="d")
            q8_i = q8p.tile([128, B], U8, tag="q8")
            nh = 2 if halves else 1
            for h in range(nh):
                wsl = slice(h * (OSH // nh), (h + 1) * (OSH // nh))
                nc.gpsimd.tensor_tensor(t_t[:, wsl], wt_i[:, wsl],
                                        rws_t[:, wsl], OP.mult)
                nc.vector.tensor_tensor(q8w[:, wsl], t_t[:, wsl],
                                        wz_t[:, wsl], OP.add)
                nc.vector.tensor_tensor(d_t[:, wsl], q8w[:, wsl],
                                        wz_t[:, wsl], OP.subtract)
                wq_v = wq_t[:, kt * OSH:(kt + 1) * OSH]
                nc.vector.tensor_tensor(wq_v[:, wsl], d_t[:, wsl],
                                        ws_t[:, wsl], OP.mult)
            # --- x chain on ACT: q8 = u8(x*(1/sa)+za); qx = bf16(q8*sa-za*sa)
            for h in range(nh):
                xsl = slice(h * (B // nh), (h + 1) * (B // nh))
                nc.scalar.activation(q8_i[:, xsl], xr_i[:, xsl], AF.Identity,
                                     bias=az_t[:, oct_:oct_ + 1],
                                     scale=ras_t[:, oct_:oct_ + 1])
                qx_v = qx_t[:, kt * B:(kt + 1) * B]
                nc.scalar.activation(qx_v[:, xsl], q8_i[:, xsl], AF.Identity,
                                     bias=nzsa_t[:, oct_:oct_ + 1],
                                     scale=asc_t[:, oct_:oct_ + 1])

        def emit_mms(i, kt):
            if not dummy_emitted[0]:
                dk = DUMMY_STRIP
                dkt = (dk % 4) * NOCT + dk // 4
                for j in range(8 - DUMMY_NCHAIN, 8):
                    nc.tensor.transpose(pacc[j][:, 0:64].bitcast(BF16),
                                        qx_t[:, dkt * B:dkt * B + 128],
                                        ident[:])
                dummy_emitted[0] = True
            # b2-major on the final strip: the 4 b2=0 matmuls only need the
            # first x-half, so they start one ACT half-op earlier
            order = ([(ot, b2) for b2 in range(2) for ot in range(NOT)]
                     if i == NKT - 1 else
                     [(ot, b2) for ot in range(NOT) for b2 in range(2)])
            for ot, b2 in order:
                lhsT = wq_t[:, kt * OSH + 128 * ot:kt * OSH + 128 * (ot + 1)]
                rhs = qx_t[:, kt * B + 512 * b2:kt * B + 512 * (b2 + 1)]
                nc.tensor.matmul(pacc[ot * 2 + b2][:], lhsT, rhs,
                                 start=(i == 0), stop=(i == NKT - 1))

        sc = emit_scales(0)
        for oct_ in range(NOCT):
            cur = sc
            for r in range(4):
                i = oct_ * 4 + r
                emit_strip(i, oct_, r, cur,
                           wx0=(wt0, xr0) if i == 0 else None,
                           halves=(i == NKT - 1))
                if r == 0 and oct_ + 1 < NOCT:
                    sc = emit_scales(oct_ + 1)
                if FILLERS and i >= FILL_FROM:
                    emit_fillers(FILLERS)
                if DUMMY_STRIP is None or i >= DUMMY_STRIP:
                    for j in (range(i + 1) if (DUMMY_STRIP is not None
                                               and i == DUMMY_STRIP)
                              else (i,)):
                        emit_mms(j, (j % 4) * NOCT + j // 4)

        # ---- drain: ACT/DVE drain the two halves of each ot in parallel,
        #      one out DMA per ot (HWDGE cost dominates the out cadence) ----
        for ot in range(NOT):
            ob = outp.tile([128, B], out_dt, tag="ob")
            nc.scalar.activation(ob[:, 0:512], pacc[ot * 2][:], AF.Identity,
                                 bias=bias_t[:, ot:ot + 1], scale=1.0)
            nc.vector.tensor_scalar(ob[:, 512:B], pacc[ot * 2 + 1][:],
                                    bias_t[:, ot:ot + 1], None, OP.add)
            nc.sync.dma_start(out_d[128 * ot:128 * (ot + 1), :], ob[:])

    nc.compile()
    return nc


def _get_nc():
    if "nc" not in _CACHE:
        _CACHE["nc"] = _build_nc()
    return _CACHE["nc"]


def _prep_inputs(x, weight, bias, w_scales, w_zeros, a_scales, a_zeros):
    """Host-side shard/layout prep. Pure slicing/permutation, no arithmetic."""
    x = np.ascontiguousarray(x, np.float32)
    # xT[r*NB + kb, b] = x[b, kb*BS + r]
    xT = np.ascontiguousarray(
        x.reshape(B, NB, BS).transpose(2, 1, 0).reshape(IN, B))
    asc2 = np.asarray(a_scales, np.float32).reshape(NOCT, 128).T
    az2 = np.asarray(a_zeros, np.float32).reshape(NOCT, 128).T
    in_maps = []
    for c in range(NCORES):
        sl = slice(c * OSH, (c + 1) * OSH)
        wsh = np.asarray(weight[sl], np.float32)
        # wT[r*NB + kb, o] = W[o, kb*BS + r]
        wT = np.ascontiguousarray(
            wsh.reshape(OSH, NB, BS).transpose(2, 1, 0).reshape(IN, OSH))
        cst = np.concatenate(
            [asc2, az2,
             np.asarray(bias[sl], np.float32).reshape(NOT, 128).T], axis=1)
        in_maps.append({
            "xT": xT,
            "wT": wT,
            "wsT": np.ascontiguousarray(
                np.asarray(w_scales[sl], np.float32).T),
            "wzT": np.ascontiguousarray(
                np.asarray(w_zeros[sl], np.float32).T),
            "cst": np.ascontiguousarray(cst),
        })
    return in_maps


def kernel(x, weight, bias, w_scales, w_zeros, a_scales, a_zeros, _res_out=None):
    nc = _get_nc()
    in_maps = _prep_inputs(x, weight, bias, w_scales, w_zeros, a_scales, a_zeros)
    res = run_bass_kernel_spmd(nc, in_maps, core_ids=list(range(NCORES)))
    if _res_out is not None:
        _res_out.append(res)
    outT = np.concatenate([np.asarray(res.results[c]["out"], np.float32)
                           for c in range(NCORES)], axis=0)
    return np.ascontiguousarray(outT.T)



# revision 23
# speedup vs baseline: 1.0046x; 1.0046x over previous
"""Trainium2 Bass kernel for nn_LinearPerBlockQuant (per-block fake-quant linear).

  out = fake_quant(x; a_scales, a_zeros) @ fake_quant(W; w_scales, w_zeros).T + bias

Shapes: x (1024, 4096) f32, W (4096, 4096), block size 4 along IN,
w_scales/w_zeros (4096, 1024), a_scales/a_zeros (1024,), bias (4096,).

Sharding: column-parallel over 8 NeuronCores -- each core owns 512 output
features (W rows, scales, bias shards); x is replicated. Host concatenates
the 8 (512, 1024) partial outputs and transposes.

Device-side per core (strip-streaming design):
  - x and W both arrive pre-transposed + block-permuted on the k axis:
      xT[r*1024+kb, b] = x[b, 4*kb+r];  wT[r*1024+kb, o] = W[o, 4*kb+r]
    so k is the partition dim everywhere and per-k activation quant
    scales are per-partition scalars (ACT scale/bias fusion). Weight
    scales arrive transposed (wsT/wzT (1024, 512)), so in a k-strip the
    per-(o, block) scales are dense (128, 512) tensor operands shared by
    the 4 strips of one kb-octave (prefetched one octave ahead).
  - quant: q = sat_u8(v * (1/s) + z)  (HW u8 conversion = round-half-even +
    saturate == clip(round(.), 0, 255), HW-verified)
  - x path on ACT: f32 -> u8 (quant), u8 -> bf16 (dequant), both with
    per-partition scale/bias fusion. W path: gpsimd mult + DVE add/sub/mult,
    result bf16. bf16 keeps full qx (64KB/part) + wqT (32KB/part) resident.
  - matmul: 8 psum tiles (128, 512) = all 8 banks, one accumulation chain
    per (ot, b-half), accumulated strip-by-strip as data lands (bf16 =
    1 cycle/row).
  - drain: bias added on psum drain, alternating ACT/DVE so the 8 drains
    run in parallel pairs; output written bf16 (halves output DMA), one
    DMA per (ot, b-half) fired straight after its drain.
"""
import os
import numpy as np
from contextlib import ExitStack

import concourse.bass as bass
import concourse.tile as tile
from concourse import bacc, mybir
from concourse.bass_utils import run_bass_kernel_spmd
from concourse.masks import make_identity

F32 = mybir.dt.float32
BF16 = mybir.dt.bfloat16
U8 = mybir.dt.uint8
OP = mybir.AluOpType
AF = mybir.ActivationFunctionType

B, IN, OUT, BS = 1024, 4096, 4096, 4
NCORES = 8
OSH = OUT // NCORES          # 512 out-features per core
NB = IN // BS                # 1024 blocks along IN
NKT = IN // 128              # 32 k-strips of 128
NOCT = 8                     # kb-octaves (128 kb values each)
NOT = OSH // 128             # 4 output-feature tiles per core
OUT_BF16 = True              # write output as bf16 (halves output DMA)
# hold PE back until strip DUMMY_STRIP's qx is ready; DUMMY_NCHAIN of the 8
# psum chains are held (dummy transpose creates a WAR dep on the psum tile)
DUMMY_STRIP = int(os.environ.get("LPBQ_DUMMY_STRIP", "-1"))
DUMMY_NCHAIN = int(os.environ.get("LPBQ_DUMMY_NCHAIN", "1"))
if DUMMY_STRIP < 0:
    DUMMY_STRIP = None
# dep-free add-zero matmuls emitted before each strip's real matmuls: they
# run while PE would otherwise starve, so the p-state ramp never resets and
# the (PE-paced) endgame runs at full clock instead of 1.2GHz
FILLERS = int(os.environ.get("LPBQ_FILLERS", "0"))
FILL_FROM = int(os.environ.get("LPBQ_FILL_FROM", "26"))
DVE_DEQ_FROM = int(os.environ.get("LPBQ_DVE_DEQ_FROM", "30"))
NCST = 2 * NOCT + NOT        # asc | az | bias columns

_CACHE = {}


def _build_nc():
    nc = bacc.Bacc("TRN2", target_bir_lowering=False, debug=False)

    xT_d = nc.dram_tensor("xT", [IN, B], F32, kind="ExternalInput").ap()
    wT_d = nc.dram_tensor("wT", [IN, OSH], F32, kind="ExternalInput").ap()
    wsT_d = nc.dram_tensor("wsT", [NB, OSH], F32, kind="ExternalInput").ap()
    wzT_d = nc.dram_tensor("wzT", [NB, OSH], F32, kind="ExternalInput").ap()
    cst_d = nc.dram_tensor("cst", [128, NCST], F32, kind="ExternalInput").ap()
    out_dt = BF16 if OUT_BF16 else F32
    out_d = nc.dram_tensor("out", [OSH, B], out_dt, kind="ExternalOutput").ap()

    with tile.TileContext(nc) as tc, ExitStack() as ctx:
        const = ctx.enter_context(tc.tile_pool(name="const", bufs=1))
        big = ctx.enter_context(tc.tile_pool(name="big", bufs=1))
        xrp = ctx.enter_context(tc.tile_pool(name="xr", bufs=5))
        q8p = ctx.enter_context(tc.tile_pool(name="q8", bufs=3))
        wtp = ctx.enter_context(tc.tile_pool(name="wt", bufs=4))
        wsp = ctx.enter_context(tc.tile_pool(name="wsp", bufs=3))
        wzp = ctx.enter_context(tc.tile_pool(name="wzp", bufs=3))
        rwsp = ctx.enter_context(tc.tile_pool(name="rws", bufs=3))
        tdp = ctx.enter_context(tc.tile_pool(name="td", bufs=4))
        q8wp = ctx.enter_context(tc.tile_pool(name="q8w", bufs=3))
        outp = ctx.enter_context(tc.tile_pool(name="outp", bufs=4))
        psm = ctx.enter_context(tc.tile_pool(name="psm", bufs=1, space="PSUM"))

        # ---- first strip's big DMAs before anything small: fill the pipe ----
        wt0 = wtp.tile([128, OSH], F32, tag="wt")
        nc.sync.dma_start(wt0[:], wT_d[0:128, :])
        xr0 = xrp.tile([128, B], F32, tag="xr")
        nc.sync.dma_start(xr0[:], xT_d[0:128, :])

        # dummy activation with no data deps: hoists the implicit
        # LoadActFuncSet (1.28us) to t~0 instead of before the first quant
        scr_t = const.tile([128, 1], F32)
        nc.vector.memset(scr_t[:], 0.0)
        nc.scalar.activation(scr_t[:], scr_t[:], AF.Identity,
                             bias=0.0, scale=1.0)

        # ---- constants: asc | az | bias in one DMA ----
        cst_t = const.tile([128, NCST], F32)
        nc.sync.dma_start(cst_t[:], cst_d)
        asc_t = cst_t[:, 0:NOCT]
        az_t = cst_t[:, NOCT:2 * NOCT]
        bias_t = cst_t[:, 2 * NOCT:]
        ras_t = const.tile([128, NOCT], F32)
        nc.vector.reciprocal(ras_t[:], asc_t)
        # nzsa = -(za * sa)
        nzsa_t = const.tile([128, NOCT], F32)
        nc.vector.scalar_tensor_tensor(nzsa_t[:], az_t, -1.0, asc_t,
                                       OP.mult, OP.mult)
        ident = None
        if DUMMY_STRIP is not None:
            ident = const.tile([128, 128], BF16)
            make_identity(nc, ident[:])

        # ---- resident big tensors ----
        qx_t = big.tile([128, NKT * B], BF16)     # dequant activations
        wq_t = big.tile([128, NKT * OSH], BF16)   # dequant transposed weights

        # 8 psum accumulators: (ot, b-half), each (128, 512) = one bank
        pacc = [psm.tile([128, 512], F32, name=f"pacc{j}") for j in range(8)]

        z_t = None
        if FILLERS:
            z_t = const.tile([128, 512], BF16)
            nc.vector.memset(z_t[:], 0.0)

        dummy_emitted = [DUMMY_STRIP is None]
        fill_cnt = [0]

        def emit_fillers(n):
            for _ in range(n):
                j = fill_cnt[0] % 8
                fill_cnt[0] += 1
                nc.tensor.matmul(pacc[j][:], z_t[:, 0:128], z_t[:],
                                 start=False, stop=False)

        def emit_scales(oct_):
            ws_t = wsp.tile([128, OSH], F32, tag="ws")
            nc.sync.dma_start(ws_t[:], wsT_d[128 * oct_:128 * (oct_ + 1), :])
            wz_t = wzp.tile([128, OSH], F32, tag="wz")
            nc.sync.dma_start(wz_t[:], wzT_d[128 * oct_:128 * (oct_ + 1), :])
            rws_t = rwsp.tile([128, OSH], F32, tag="rws")
            nc.vector.reciprocal_approx_fast(rws_t[:], ws_t[:])
            return ws_t, wz_t, rws_t

        def emit_strip(i, oct_, r, scales, wx0=None, halves=False):
            kt = r * NOCT + oct_
            ws_t, wz_t, rws_t = scales
            # --- DMAs (w first: its chain is one hop longer) ---
            if wx0 is not None:
                wt_i, xr_i = wx0
            elif halves:
                # split the final strip's DMAs so each half lands (and its
                # dependent chain starts) one half-transfer earlier
                wt_i = wtp.tile([128, OSH], F32, tag="wt")
                xr_i = xrp.tile([128, B], F32, tag="xr")
                for h in range(2):
                    ws_ = slice(h * (OSH // 2), (h + 1) * (OSH // 2))
                    nc.sync.dma_start(wt_i[:, ws_],
                                      wT_d[128 * kt:128 * (kt + 1), ws_])
                for h in range(2):
                    xs_ = slice(h * (B // 2), (h + 1) * (B // 2))
                    nc.sync.dma_start(xr_i[:, xs_],
                                      xT_d[128 * kt:128 * (kt + 1), xs_])
            else:
                wt_i = wtp.tile([128, OSH], F32, tag="wt")
                nc.sync.dma_start(wt_i[:], wT_d[128 * kt:128 * (kt + 1), :])
                xr_i = xrp.tile([128, B], F32, tag="xr")
                nc.sync.dma_start(xr_i[:], xT_d[128 * kt:128 * (kt + 1), :])
            # --- W chain: t = w*rws (Pool); q8w = u8(t+wz); d = q8w-wz;
            #     wq = bf16(d*ws) --- (half-split on the last strip so the
            # tail-chain latency after the final DMA is ~halved)
            t_t = tdp.tile([128, OSH], F32, tag="t")
            q8w = q8wp.tile([128, OSH], U8, tag="q8w")
            d_t = tdp.tile([128, OSH], F32, tag="d")
            q8_i = q8p.tile([128, B], U8, tag="q8")
            nh = 2 if halves else 1
            for h in range(nh):
                wsl = slice(h * (OSH // nh), (h + 1) * (OSH // nh))
                nc.gpsimd.tensor_tensor(t_t[:, wsl], wt_i[:, wsl],
                                        rws_t[:, wsl], OP.mult)
                nc.vector.tensor_tensor(q8w[:, wsl], t_t[:, wsl],
                                        wz_t[:, wsl], OP.add)
                nc.vector.tensor_tensor(d_t[:, wsl], q8w[:, wsl],
                                        wz_t[:, wsl], OP.subtract)
                wq_v = wq_t[:, kt * OSH:(kt + 1) * OSH]
                nc.vector.tensor_tensor(wq_v[:, wsl], d_t[:, wsl],
                                        ws_t[:, wsl], OP.mult)
            # --- x chain on ACT: q8 = u8(x*(1/sa)+za); qx = bf16(q8*sa-za*sa)
            deq_dve = i >= DVE_DEQ_FROM
            for h in range(nh):
                xsl = slice(h * (B // nh), (h + 1) * (B // nh))
                nc.scalar.activation(q8_i[:, xsl], xr_i[:, xsl], AF.Identity,
                                     bias=az_t[:, oct_:oct_ + 1],
                                     scale=ras_t[:, oct_:oct_ + 1])
                qx_v = qx_t[:, kt * B:(kt + 1) * B]
                if deq_dve:
                    nc.vector.tensor_scalar(qx_v[:, xsl], q8_i[:, xsl],
                                            asc_t[:, oct_:oct_ + 1],
                                            nzsa_t[:, oct_:oct_ + 1],
                                            OP.mult, OP.add)
                else:
                    nc.scalar.activation(qx_v[:, xsl], q8_i[:, xsl],
                                         AF.Identity,
                                         bias=nzsa_t[:, oct_:oct_ + 1],
                                         scale=asc_t[:, oct_:oct_ + 1])

        def emit_mms(i, kt):
            if not dummy_emitted[0]:
                dk = DUMMY_STRIP
                dkt = (dk % 4) * NOCT + dk // 4
                for j in range(8 - DUMMY_NCHAIN, 8):
                    nc.tensor.transpose(pacc[j][:, 0:64].bitcast(BF16),
                                        qx_t[:, dkt * B:dkt * B + 128],
                                        ident[:])
                dummy_emitted[0] = True
            # b2-major on the final strip: the 4 b2=0 matmuls only need the
            # first x-half, so they start one ACT half-op earlier
            order = ([(ot, b2) for b2 in range(2) for ot in range(NOT)]
                     if i == NKT - 1 else
                     [(ot, b2) for ot in range(NOT) for b2 in range(2)])
            for ot, b2 in order:
                lhsT = wq_t[:, kt * OSH + 128 * ot:kt * OSH + 128 * (ot + 1)]
                rhs = qx_t[:, kt * B + 512 * b2:kt * B + 512 * (b2 + 1)]
                nc.tensor.matmul(pacc[ot * 2 + b2][:], lhsT, rhs,
                                 start=(i == 0), stop=(i == NKT - 1))

        sc = emit_scales(0)
        for oct_ in range(NOCT):
            cur = sc
            for r in range(4):
                i = oct_ * 4 + r
                emit_strip(i, oct_, r, cur,
                           wx0=(wt0, xr0) if i == 0 else None,
                           halves=(i == NKT - 1))
                if r == 0 and oct_ + 1 < NOCT:
                    sc = emit_scales(oct_ + 1)
                if FILLERS and i >= FILL_FROM:
                    emit_fillers(FILLERS)
                if DUMMY_STRIP is None or i >= DUMMY_STRIP:
                    for j in (range(i + 1) if (DUMMY_STRIP is not None
                                               and i == DUMMY_STRIP)
                              else (i,)):
                        emit_mms(j, (j % 4) * NOCT + j // 4)

        # ---- drain: ACT/DVE drain the two halves of each ot in parallel,
        #      one out DMA per ot (HWDGE cost dominates the out cadence) ----
        for ot in range(NOT):
            ob = outp.tile([128, B], out_dt, tag="ob")
            nc.scalar.activation(ob[:, 0:512], pacc[ot * 2][:], AF.Identity,
                                 bias=bias_t[:, ot:ot + 1], scale=1.0)
            nc.vector.tensor_scalar(ob[:, 512:B], pacc[ot * 2 + 1][:],
                                    bias_t[:, ot:ot + 1], None, OP.add)
            nc.sync.dma_start(out_d[128 * ot:128 * (ot + 1), :], ob[:])

    nc.compile()
    return nc


def _get_nc():
    if "nc" not in _CACHE:
        _CACHE["nc"] = _build_nc()
    return _CACHE["nc"]


def _prep_inputs(x, weight, bias, w_scales, w_zeros, a_scales, a_zeros):
    """Host-side shard/layout prep. Pure slicing/permutation, no arithmetic."""
    x = np.ascontiguousarray(x, np.float32)
    # xT[r*NB + kb, b] = x[b, kb*BS + r]
    xT = np.ascontiguousarray(
        x.reshape(B, NB, BS).transpose(2, 1, 0).reshape(IN, B))
    asc2 = np.asarray(a_scales, np.float32).reshape(NOCT, 128).T
    az2 = np.asarray(a_zeros, np.float32).reshape(NOCT, 128).T
    in_maps = []
    for c in range(NCORES):
        sl = slice(c * OSH, (c + 1) * OSH)
        wsh = np.asarray(weight[sl], np.float32)
        # wT[r*NB + kb, o] = W[o, kb*BS + r]
        wT = np.ascontiguousarray(
            wsh.reshape(OSH, NB, BS).transpose(2, 1, 0).reshape(IN, OSH))
        cst = np.concatenate(
            [asc2, az2,
             np.asarray(bias[sl], np.float32).reshape(NOT, 128).T], axis=1)
        in_maps.append({
            "xT": xT,
            "wT": wT,
            "wsT": np.ascontiguousarray(
                np.asarray(w_scales[sl], np.float32).T),
            "wzT": np.ascontiguousarray(
                np.asarray(w_zeros[sl], np.float32).T),
            "cst": np.ascontiguousarray(cst),
        })
    return in_maps


def kernel(x, weight, bias, w_scales, w_zeros, a_scales, a_zeros, _res_out=None):
    nc = _get_nc()
    in_maps = _prep_inputs(x, weight, bias, w_scales, w_zeros, a_scales, a_zeros)
    res = run_bass_kernel_spmd(nc, in_maps, core_ids=list(range(NCORES)))
    if _res_out is not None:
        _res_out.append(res)
    outT = np.concatenate([np.asarray(res.results[c]["out"], np.float32)
                           for c in range(NCORES)], axis=0)
    return np.ascontiguousarray(outT.T)

